# revision 1
# baseline (speedup 1.0000x reference)
"""GAT layer (nn_GATLayer_53841710022826) Bass/Tile kernel for Trainium2.

Data-parallel across 8 NeuronCores: each core processes a contiguous shard of
the agent dimension.  All weights are replicated.

Math (per agent n, heads h=4, mid m=32, neighbors k=4):
  a  = relu(agent @ W_agent + b_agent)            [n, 4, 32]
  nr = relu(neighbor @ W_neigh + b_neigh)         [n, 4, 4, 32]
  nh = relu(neighbor @ W_hid + b_hid)             [n, 4, 4, 32]
  att[n,h,k]   = sum_m a[n,h,m] * nr[n,k,h,m]
  score        = softmax_k(att - 1e8*mask)        (exactly 0 for masked k)
  out[n,h,m]   = sum_k score[n,h,k] * nh[n,k,h,m] / 4
  y  = relu(out @ W_out + b_out)                  [n, 64]

Kernel layout choice: agents on the partition dimension everywhere.  The
PE transposes each activation chunk ([P,64] -> [64,P]) so the transposed
activations act as the (self-loading f32) stationary matmul operand with the
small weight matrices as the moving operand; projections land in PSUM in
agent-major layout, where relu/attention/softmax are all free-dim vector ops.
"""

import os
import sys

if "/opt/trn_rl_repo" not in sys.path:
    sys.path.insert(0, "/opt/trn_rl_repo")

from contextlib import ExitStack

import numpy as np

import concourse.bass as bass
import concourse.tile as tile
from concourse import bacc, masks, mybir
from concourse.bass_utils import run_bass_kernel_spmd

F32 = mybir.dt.float32
I32 = mybir.dt.int32
AF = mybir.ActivationFunctionType
ALU = mybir.AluOpType
AX = mybir.AxisListType

N_CORES = 8
AGENT_SIZE = 64
NEIGH_SIZE = 64
NUM_HEAD = 4
MID_SIZE = 32
NK = 4
HM = NUM_HEAD * MID_SIZE  # 128
OUT = HM // 2  # 64

_BUILD_CACHE = {}


def _emit_chunk(ctx, tc, pools, aps, c0, P, has_bias):
    """Emit instructions for one chunk of P (<=128) agents starting at row c0."""
    nc = tc.nc
    (inp, xt_ps, xt_sb, proj_ps, work, out_ps, const) = pools
    (agent, neigh, mask, y, wa, wn, wh, wo4, bias_abc, bias_o, ident) = aps

    # ---- load inputs (f32) ----
    ag = inp.tile([128, AGENT_SIZE], F32, tag="ag")
    nc.sync.dma_start(ag[:P], agent[c0 : c0 + P, :])
    nb = inp.tile([128, NK * NEIGH_SIZE], F32, tag="nb")
    nc.sync.dma_start(nb[:P], neigh[c0 : c0 + P, :])
    mk = inp.tile([128, NK], I32, tag="mk")
    nc.sync.dma_start(mk[:P], mask[c0 : c0 + P, :])

    # ---- transpose activations: [P, 64] -> [64, P] (PE + PSUM->SBUF copy) ----
    # One PSUM bank tile holds agentT + neighT slots 0..2, a second holds slot 3.
    t1 = xt_ps.tile([64, 512], F32, tag="t1")
    t2 = xt_ps.tile([64, 128], F32, tag="t2")
    nc.tensor.transpose(t1[:, 0:P], ag[:P, :], ident[:P, :P])
    for k in range(3):
        nc.tensor.transpose(
            t1[:, 128 * (k + 1) : 128 * (k + 1) + P],
            nb[:P, 64 * k : 64 * (k + 1)],
            ident[:P, :P],
        )
    nc.tensor.transpose(t2[:, 0:P], nb[:P, 64 * 3 : 64 * 4], ident[:P, :P])

    xt = xt_sb.tile([64, 640], F32, tag="xt")  # [agT | nbT0..3]
    nc.scalar.activation(xt[:, 0:512], t1[:, :], AF.Copy)
    nc.scalar.activation(xt[:, 512:640], t2[:, :], AF.Copy)
    agT = lambda: xt[:, 0:P]
    nbT = lambda k: xt[:, 128 * (k + 1) : 128 * (k + 1) + P]

    # ---- projections: out[P, 128] = x @ W  (lhsT = xT, rhs = W) ----
    pa = proj_ps.tile([128, 512], F32, tag="pa")  # [a | nr0 | nr1 | nr2]
    pb = proj_ps.tile([128, 512], F32, tag="pb")  # [nr3 | nh0 | nh1 | nh2]
    pc = out_ps.tile([128, 320], F32, tag="pc")  # [nh3 | oT | y]
    nc.tensor.matmul(pa[:P, 0:128], agT(), wa[:, :])
    for k in range(4):
        dst = pa[:P, 128 * (k + 1) : 128 * (k + 2)] if k < 3 else pb[:P, 0:128]
        nc.tensor.matmul(dst, nbT(k), wn[:, :])
    for k in range(4):
        dst = pb[:P, 128 * (k + 1) : 128 * (k + 2)] if k < 3 else pc[:P, 0:128]
        nc.tensor.matmul(dst, nbT(k), wh[:, :])

    a_psv = pa[:P, 0:128]
    nr_psv = [
        pa[:P, 128:256],
        pa[:P, 256:384],
        pa[:P, 384:512],
        pb[:P, 0:128],
    ]
    nh_psv = [
        pb[:P, 128:256],
        pb[:P, 256:384],
        pb[:P, 384:512],
        pc[:P, 0:128],
    ]

    # ---- relu (PSUM -> SBUF); pa read only by DVE, pb/pc only by ACT ----
    # (single reader engine per PSUM tile keeps matmul sync-wait counts low)
    ba = bias_abc
    a_r = work.tile([128, HM], F32, tag="a_r")
    nr_r = work.tile([128, 4 * HM], F32, tag="nr_r")
    nh_r = work.tile([128, 4 * HM], F32, tag="nh_r")
    if has_bias:
        # out = relu(in * 1 + bias); ACT bias is per-partition, so biases are
        # handled with tensor_tensor add + max instead.
        nc.vector.tensor_tensor(a_psv, a_psv, ba[:P, 0:128], op=ALU.add)
        for k in range(4):
            nc.vector.tensor_tensor(nr_psv[k], nr_psv[k], ba[:P, 128:256], op=ALU.add)
            nc.vector.tensor_tensor(nh_psv[k], nh_psv[k], ba[:P, 256:384], op=ALU.add)
    nc.vector.tensor_scalar_max(a_r[:P], a_psv, 0.0)
    for k in range(4):
        dst_nr = nr_r[:P, 128 * k : 128 * (k + 1)]
        dst_nh = nh_r[:P, 128 * k : 128 * (k + 1)]
        if k < 3:
            nc.vector.tensor_scalar_max(dst_nr, nr_psv[k], 0.0)
            nc.scalar.activation(dst_nh, nh_psv[k], AF.Relu)
        else:
            nc.scalar.activation(dst_nr, nr_psv[k], AF.Relu)
            nc.scalar.activation(dst_nh, nh_psv[k], AF.Relu)

    # ---- attention logits: att[:, 4h+k] = sum_m a_r[:, hm] * nr_r_k[:, hm] ----
    att = work.tile([128, NUM_HEAD * NK], F32, tag="att")
    prod = work.tile([128, HM], F32, tag="prod")
    att_v = att[:P].rearrange("p (h k) -> p h k", k=NK)
    for k in range(4):
        nc.vector.tensor_tensor(
            prod[:P], a_r[:P], nr_r[:P, 128 * k : 128 * (k + 1)], op=ALU.mult
        )
        nc.vector.tensor_reduce(
            att_v[:, :, k],
            prod[:P].rearrange("p (h m) -> p h m", h=NUM_HEAD),
            axis=AX.X,
            op=ALU.add,
        )

    # ---- mask penalty + softmax over k (free-dim ops, agents on partitions) ----
    mkp = work.tile([128, NK], F32, tag="mkp")
    nc.vector.tensor_scalar_mul(mkp[:P], mk[:P], -1.0e8)
    attm = work.tile([128, NUM_HEAD * NK], F32, tag="attm")
    # broadcast mask penalty over heads: mkp [P, k] -> [P, h, k]
    mkp_b = mkp[:P].unsqueeze(1).broadcast_to([P, NUM_HEAD, NK])
    nc.vector.tensor_tensor(
        attm[:P].rearrange("p (h k) -> p h k", k=NK), att_v, mkp_b, op=ALU.add
    )
    es = work.tile([128, NUM_HEAD * NK], F32, tag="es")
    nc.scalar.activation(es[:P], attm[:P], AF.Exp)
    s4 = work.tile([128, NUM_HEAD], F32, tag="s4")
    nc.vector.tensor_reduce(
        s4[:P], es[:P].rearrange("p (h k) -> p h k", k=NK), axis=AX.X, op=ALU.add
    )
    s4m = work.tile([128, NUM_HEAD], F32, tag="s4m")
    nc.vector.tensor_scalar_max(s4m[:P], s4[:P], 1.0e-30)
    r4 = work.tile([128, NUM_HEAD], F32, tag="r4")
    nc.vector.reciprocal(r4[:P], s4m[:P])
    score = work.tile([128, NUM_HEAD * NK], F32, tag="score")
    r4_b = r4[:P].unsqueeze(2).broadcast_to([P, NUM_HEAD, NK])
    nc.vector.tensor_tensor(
        score[:P].rearrange("p (h k) -> p h k", k=NK),
        es[:P].rearrange("p (h k) -> p h k", k=NK),
        r4_b,
        op=ALU.mult,
    )

    # ---- weighted sum over neighbors: outacc[:, hm] = sum_k score_k * nh_r_k ----
    wk01 = work.tile([128, HM], F32, tag="wk01")
    wk23 = work.tile([128, HM], F32, tag="wk23")
    wkt = work.tile([128, HM], F32, tag="wkt")
    outacc = work.tile([128, HM], F32, tag="outacc")
    sc_v = score[:P].rearrange("p (h k) -> p h k", k=NK)

    def score_k(k):
        # [P, h] -> broadcast over m -> [P, h, m]
        return sc_v[:, :, k].unsqueeze(2).broadcast_to([P, NUM_HEAD, MID_SIZE])

    def nh_k(k):
        return nh_r[:P, 128 * k : 128 * (k + 1)].rearrange(
            "p (h m) -> p h m", h=NUM_HEAD
        )

    ge = nc.gpsimd
    ge.tensor_tensor(
        wk01[:P].rearrange("p (h m) -> p h m", h=NUM_HEAD), score_k(0), nh_k(0), op=ALU.mult
    )
    ge.tensor_tensor(
        wkt[:P].rearrange("p (h m) -> p h m", h=NUM_HEAD), score_k(1), nh_k(1), op=ALU.mult
    )
    ge.tensor_tensor(wk01[:P], wk01[:P], wkt[:P], op=ALU.add)
    ge.tensor_tensor(
        wk23[:P].rearrange("p (h m) -> p h m", h=NUM_HEAD), score_k(2), nh_k(2), op=ALU.mult
    )
    ge.tensor_tensor(
        wkt[:P].rearrange("p (h m) -> p h m", h=NUM_HEAD), score_k(3), nh_k(3), op=ALU.mult
    )
    ge.tensor_tensor(wk23[:P], wk23[:P], wkt[:P], op=ALU.add)
    ge.tensor_tensor(outacc[:P], wk01[:P], wk23[:P], op=ALU.add)

    # ---- output projection: y = relu(outacc @ (W_out/4) + b_out) ----
    oT_ps = pc[:, 128:256]
    nc.tensor.transpose(oT_ps[:, 0:P], outacc[:P, :], ident[:P, :P])
    oT = work.tile([128, 128], F32, tag="oTsb")
    nc.scalar.activation(oT[:, 0:P], oT_ps[:, 0:P], AF.Copy)
    y_ps = pc[:, 256:320]
    nc.tensor.matmul(y_ps[:P], oT[:, 0:P], wo4[:, :])
    if has_bias:
        nc.vector.tensor_tensor(y_ps[:P], y_ps[:P], bias_o[:P, :], op=ALU.add)
    y_r = work.tile([128, OUT], F32, tag="y_r")
    nc.scalar.activation(y_r[:P], y_ps[:P], AF.Relu)
    nc.sync.dma_start(y[c0 : c0 + P, :], y_r[:P])


def _build(n_per_core, has_bias):
    key = (n_per_core, has_bias)
    if key in _BUILD_CACHE:
        return _BUILD_CACHE[key]

    nc = bacc.Bacc()
    agent = nc.dram_tensor("agent", [n_per_core, AGENT_SIZE], F32, kind="ExternalInput").ap()
    neigh = nc.dram_tensor(
        "neighbor", [n_per_core, NK * NEIGH_SIZE], F32, kind="ExternalInput"
    ).ap()
    mask = nc.dram_tensor("mask", [n_per_core, NK], I32, kind="ExternalInput").ap()
    wa = nc.dram_tensor("wa", [AGENT_SIZE, HM], F32, kind="ExternalInput").ap()
    wn = nc.dram_tensor("wn", [NEIGH_SIZE, HM], F32, kind="ExternalInput").ap()
    wh = nc.dram_tensor("wh", [NEIGH_SIZE, HM], F32, kind="ExternalInput").ap()
    wo4 = nc.dram_tensor("wo4", [HM, OUT], F32, kind="ExternalInput").ap()
    biases = nc.dram_tensor("biases", [1, 3 * HM + OUT], F32, kind="ExternalInput").ap()
    y = nc.dram_tensor("y", [n_per_core, OUT], F32, kind="ExternalOutput").ap()

    with ExitStack() as ctx:
        tc = ctx.enter_context(tile.TileContext(nc))
        const = ctx.enter_context(tc.tile_pool(name="const", bufs=1))
        inp = ctx.enter_context(tc.tile_pool(name="inp", bufs=3))
        xt_ps = ctx.enter_context(tc.tile_pool(name="xt_ps", bufs=2, space="PSUM"))
        xt_sb = ctx.enter_context(tc.tile_pool(name="xt_sb", bufs=2))
        proj_ps = ctx.enter_context(tc.tile_pool(name="proj_ps", bufs=1, space="PSUM"))
        work = ctx.enter_context(tc.tile_pool(name="work", bufs=2))
        out_ps = ctx.enter_context(tc.tile_pool(name="out_ps", bufs=2, space="PSUM"))

        # constants in SBUF
        ident = const.tile([128, 128], F32)
        masks.make_identity(nc, ident[:])
        wa_sb = const.tile([AGENT_SIZE, HM], F32)
        nc.sync.dma_start(wa_sb[:], wa[:, :])
        wn_sb = const.tile([NEIGH_SIZE, HM], F32)
        nc.sync.dma_start(wn_sb[:], wn[:, :])
        wh_sb = const.tile([NEIGH_SIZE, HM], F32)
        nc.sync.dma_start(wh_sb[:], wh[:, :])
        wo4_sb = const.tile([HM, OUT], F32)
        nc.sync.dma_start(wo4_sb[:], wo4[:, :])
        bias_abc = None
        bias_o = None
        if has_bias:
            bias_abc = const.tile([128, 3 * HM], F32)
            nc.sync.dma_start(
                bias_abc[:], biases[0:1, 0 : 3 * HM].broadcast_to([128, 3 * HM])
            )
            bias_o = const.tile([128, OUT], F32)
            nc.sync.dma_start(
                bias_o[:], biases[0:1, 3 * HM :].broadcast_to([128, OUT])
            )

        pools = (inp, xt_ps, xt_sb, proj_ps, work, out_ps, const)
        aps = (agent, neigh, mask, y, wa_sb, wn_sb, wh_sb, wo4_sb, bias_abc, bias_o, ident)

        n_full, rem = divmod(n_per_core, 128)
        for c in range(n_full):
            _emit_chunk(None, tc, pools, aps, c * 128, 128, has_bias)
        if rem:
            _emit_chunk(None, tc, pools, aps, n_full * 128, rem, has_bias)

    nc.compile()
    _BUILD_CACHE[key] = nc
    return nc


BF16 = mybir.dt.bfloat16


def _emit_block_v2(tc, pools, aps, b0, has_bias, stage=99):
    """One block of 512 agents (4 chunks of 128), bf16 compute.

    Layouts:
      T-layout  [feature/hm on partitions, agents on free]  — projections, att
      A-layout  [agents on partitions, features on free]    — softmax, weighted sum
    """
    nc = tc.nc
    (inp, xtp, sbuf, psA, psT, psS) = pools
    (agent, neigh, mask, y, wst, hsel4, wo4, identb, maskc) = aps
    # wst: [128, 384] bf16 = [WaWa | WnWn | WhWh] stacked pairs;  wo4: [128, 64] bf16
    # hsel4: [128, 128] bf16 head-selector (col 32j+h = 1 for rows h*32..h*32+31 when j==k)
    CH = 4  # chunks per block

    # ---- load (cast f32 -> bf16 on SWDGE), block rows b0 .. b0+512 ----
    ag = inp.tile([128, CH * 64], BF16, tag="ag")
    nb = inp.tile([128, CH * 256], BF16, tag="nb")
    mk = inp.tile([128, CH * NK], I32, tag="mk")
    # DMA with 3D APs: partition = within-chunk row, free = (chunk, feat)
    nc.gpsimd.dma_start(
        ag[:, :], agent[b0 : b0 + 512, :].rearrange("(c p) f -> p c f", p=128)
    )
    nc.gpsimd.dma_start(
        nb[:, :], neigh[b0 : b0 + 512, :].rearrange("(c p) f -> p c f", p=128)
    )
    nc.sync.dma_start(
        mk[:, :], mask[b0 : b0 + 512, :].rearrange("(c p) k -> p c k", p=128)
    )

    # ---- input transposes on the PE (identity matmul) + PSUM->SBUF copies ----
    # (xbar dma_start_transpose costs ~1.2us/op on the Sync queue - it was the
    # kernel's critical path; PE transposes stream at ~107ns/tile instead)
    # xt_n01[:, 128c:...] rows 0-63 = k0 feats of chunk c, rows 64-127 = k1
    xt_n01 = xtp.tile([128, 512], BF16, tag="xt01")
    xt_n23 = xtp.tile([128, 512], BF16, tag="xt23")
    xt_a = xtp.tile([128, 256], BF16, tag="xta")
    t_ps1 = psT.tile([128, 1024], BF16, tag="pt", name="t_ps1")  # n01 | n23
    for c in range(CH):
        nc.tensor.transpose(
            t_ps1[:, 128 * c : 128 * (c + 1)], nb[:, 256 * c : 256 * c + 128], identb[:, :]
        )
        nc.tensor.transpose(
            t_ps1[:, 512 + 128 * c : 640 + 128 * c],
            nb[:, 256 * c + 128 : 256 * c + 256],
            identb[:, :],
        )
    nc.vector.tensor_copy(xt_n01[:, :], t_ps1[:, 0:512])
    nc.vector.tensor_copy(xt_n23[:, :], t_ps1[:, 512:1024])
    t_ps2 = psT.tile([128, 256], BF16, tag="pt", name="t_ps2")  # agent
    nc.tensor.transpose(t_ps2[:, 0:128], ag[:, 0:128], identb[:, :])
    nc.tensor.transpose(t_ps2[:, 128:256], ag[:, 128:256], identb[:, :])
    nc.scalar.activation(xt_a[:, :], t_ps2[:, :], AF.Copy)

    def _bail(t):
        w = t.shape[-1]
        y_sb = sbuf.tile([128, 256], F32, tag="y_sb")
        if w < 256:
            nc.gpsimd.memset(y_sb[:, :], 0.0)
        nc.vector.tensor_copy(y_sb[:, 0:w], t)
        nc.sync.dma_start(
            y[b0 : b0 + 512, :].rearrange("(c p) f -> p c f", p=128), y_sb[:, :]
        )

    if stage <= 1:
        _bail(xt_n01.bitcast(F32)[:, 0:256])
        return

    # ---- projections ----
    # T-layout: a [128hm, agents]; nr_k [128hm, 512a]
    # Concurrent row-tiled matmul pairs must write DIFFERENT psum banks
    # (same-bank column-disjoint concurrent PE writes conflict), so the agent
    # projection's top/bottom halves get separate tiles: a0 = (c0, c2),
    # a1 = (c1, c3).
    a_ps0 = psS.tile([128, 256], F32, tag="ps", name="a_ps0")
    a_ps1 = psS.tile([128, 256], F32, tag="ps", name="a_ps1")
    nr_ps = [psA.tile([128, 512], F32, tag="pp", name=f"nr_ps{k}") for k in range(NK)]
    for u in range(2):  # units (c0,c1), (c2,c3)
        nc.tensor.matmul(
            a_ps0[:, 128 * u : 128 * (u + 1)],
            wst[0:64, 0:128],
            xt_a[0:64, 128 * u : 128 * (u + 1)],
            tile_position=(0, 0),
        )
        nc.tensor.matmul(
            a_ps1[:, 128 * u : 128 * (u + 1)],
            wst[64:128, 0:128],
            xt_a[64:128, 128 * u : 128 * (u + 1)],
            tile_position=(64, 0),
        )

    # neighbors nr_k (T-layout), pairs (k0,k1) and (k2,k3) concurrently
    for kp in range(2):  # pair index
        xt = xt_n01 if kp == 0 else xt_n23
        nc.tensor.matmul(
            nr_ps[2 * kp][:, :], wst[0:64, 128:256], xt[0:64, :], tile_position=(0, 0)
        )
        nc.tensor.matmul(
            nr_ps[2 * kp + 1][:, :],
            wst[64:128, 128:256],
            xt[64:128, :],
            tile_position=(64, 0),
        )

    if stage <= 2:
        _bail(nr_ps[0][:, 0:256])
        return

    # ---- a_r = relu(a_ps) (ACT), prod_k = relu(nr_k) * a_r (DVE fused) ----
    a_r = sbuf.tile([128, 512], BF16, tag="a_r")
    a_r_v = a_r.rearrange("p (u c f) -> p u c f", u=2, c=2)  # (unit, chunk-in-unit, f)
    nc.scalar.activation(
        a_r_v.transpose([0, 2, 1, 3])[:, 0], a_ps0.rearrange("p (u f) -> p u f", u=2), AF.Relu
    )
    nc.scalar.activation(
        a_r_v.transpose([0, 2, 1, 3])[:, 1], a_ps1.rearrange("p (u f) -> p u f", u=2), AF.Relu
    )
    prods = []
    for k in range(NK):
        p_t = sbuf.tile([128, 512], BF16, tag=f"prod{k}")
        if k < 2:
            nc.vector.scalar_tensor_tensor(
                p_t[:, :], nr_ps[k][:, :], 0.0, a_r[:, :], op0=ALU.max, op1=ALU.mult
            )
        else:
            # split load: ACT relu + DVE mul
            nr_r = sbuf.tile([128, 512], BF16, tag=f"nr_r{k}")
            nc.scalar.activation(nr_r[:, :], nr_ps[k][:, :], AF.Relu)
            nc.vector.tensor_tensor(p_t[:, :], nr_r[:, :], a_r[:, :], op=ALU.mult)
        prods.append(p_t)

    # ---- attention logits via PE: att_ps rows 32j+h = att(k=j, h); cols agents ----
    att_ps = psS.tile([128, 512], F32, tag="ps")
    for k in range(NK):
        # M=32 (only cols 32k+h, h<4 are nonzero) so every PSUM row is written
        nc.tensor.matmul(
            att_ps[32 * k : 32 * k + 32, :],
            hsel4[:, 32 * k : 32 * k + 32],
            prods[k][:, :],
            tile_position=(0, 32 * k),
        )
    if stage <= 3:
        _bail(att_ps[:, 0:256])
        return

    att_sb = sbuf.tile([128, 512], BF16, tag="attsb")
    nc.scalar.activation(att_sb[:, :], att_ps[:, :], AF.Copy)

    # ---- transpose att to A-layout: attT[128a, (c, 32k+h)] ----
    attT_oT = psS.tile([128, 1024], BF16, tag="ps")
    attT = attT_oT[:, 0:512]
    for c in range(CH):
        nc.tensor.transpose(
            attT[:, 128 * c : 128 * (c + 1)],
            att_sb[:, 128 * c : 128 * (c + 1)],
            identb[:, :],
        )

    # ---- softmax over k (A-layout; all small free dims) ----
    mkp = sbuf.tile([128, CH * NK], F32, tag="mkp")
    nc.vector.tensor_scalar_mul(mkp[:, :], mk[:, :], -1.0e8)
    am = sbuf.tile([128, CH * 16], F32, tag="am")
    # iterate (c, k, h): in att at col [128c + 32k + h], out at (c, h, k)
    in_v = attT.rearrange("p (c r) -> p c r", c=CH)
    in_ckh = in_v.rearrange("p c (k r) -> p c k r", k=NK)[:, :, :, 0:4]
    mkp_ckh = mkp.rearrange("p (c k) -> p c k", c=CH).unsqueeze(3).broadcast_to(
        [128, CH, NK, NUM_HEAD]
    )
    am_ckh = am.rearrange("p (c h k) -> p c h k", c=CH, h=NUM_HEAD).transpose(
        [0, 1, 3, 2]
    )
    nc.vector.tensor_tensor(am_ckh, in_ckh, mkp_ckh, op=ALU.add)
    es = sbuf.tile([128, CH * 16], F32, tag="es")
    nc.scalar.activation(es[:, :], am[:, :], AF.Exp)
    ssum = sbuf.tile([128, CH * NUM_HEAD], F32, tag="ssum")
    nc.vector.tensor_reduce(
        ssum.rearrange("p (c h) -> p c h", c=CH),
        es.rearrange("p (c h k) -> p c h k", c=CH, h=NUM_HEAD),
        axis=AX.X,
        op=ALU.add,
    )
    rs = sbuf.tile([128, CH * NUM_HEAD], F32, tag="rs")
    nc.vector.tensor_scalar_max(ssum[:, :], ssum[:, :], 1.0e-30)
    nc.vector.reciprocal(rs[:, :], ssum[:, :])
    score = sbuf.tile([128, CH * 16], BF16, tag="score")
    nc.vector.tensor_tensor(
        score.rearrange("p (c h k) -> p c h k", c=CH, h=NUM_HEAD),
        es.rearrange("p (c h k) -> p c h k", c=CH, h=NUM_HEAD),
        rs.rearrange("p (c h) -> p c h", c=CH).unsqueeze(3).broadcast_to(
            [128, CH, NUM_HEAD, NK]
        ),
        op=ALU.mult,
    )

    if stage <= 4:
        _bail(es[:, :])
        return

    # ---- nh projections in A-layout: lhsT = xt unit (k-pair), rhs = Wh ----
    nh_ps = [psA.tile([128, 512], F32, tag="pp", name=f"nh_ps{k}") for k in range(NK)]
    for c in range(CH):
        for kp in range(2):
            xt = xt_n01 if kp == 0 else xt_n23
            nc.tensor.matmul(
                nh_ps[2 * kp][:, 128 * c : 128 * (c + 1)],
                xt[0:64, 128 * c : 128 * (c + 1)],
                wst[0:64, 256:384],
                tile_position=(0, 0),
            )
            nc.tensor.matmul(
                nh_ps[2 * kp + 1][:, 128 * c : 128 * (c + 1)],
                xt[64:128, 128 * c : 128 * (c + 1)],
                wst[64:128, 256:384],
                tile_position=(64, 0),
            )

    # ---- weighted sum: wk_k = relu(nh_k) * score_k (A-layout, fused) ----
    wks = []
    for k in range(NK):
        wk = sbuf.tile([128, 512], BF16, tag=f"wk{k}")
        sc_v = (
            score.rearrange("p (c h k) -> p c h k", c=CH, h=NUM_HEAD)[:, :, :, k]
            .unsqueeze(3)
            .broadcast_to([128, CH, NUM_HEAD, MID_SIZE])
        )
        nh_v = nh_ps[k].rearrange("p (c h m) -> p c h m", c=CH, h=NUM_HEAD)
        wk_v = wk.rearrange("p (c h m) -> p c h m", c=CH, h=NUM_HEAD)
        if k < 2:
            nc.vector.scalar_tensor_tensor(
                wk_v, nh_v, 0.0, sc_v, op0=ALU.max, op1=ALU.mult
            )
        else:
            nh_r = sbuf.tile([128, 512], BF16, tag=f"nh_r{k}")
            nc.scalar.activation(nh_r[:, :], nh_ps[k][:, :], AF.Relu)
            nc.gpsimd.tensor_tensor(
                wk_v, nh_r.rearrange("p (c h m) -> p c h m", c=CH, h=NUM_HEAD), sc_v,
                op=ALU.mult,
            )
        wks.append(wk)

    u01 = sbuf.tile([128, 512], BF16, tag="u01")
    u23 = sbuf.tile([128, 512], BF16, tag="u23")
    outacc = sbuf.tile([128, 512], BF16, tag="outacc")
    nc.gpsimd.tensor_tensor(u01[:, :], wks[0][:, :], wks[1][:, :], op=ALU.add)
    nc.gpsimd.tensor_tensor(u23[:, :], wks[2][:, :], wks[3][:, :], op=ALU.add)
    nc.gpsimd.tensor_tensor(outacc[:, :], u01[:, :], u23[:, :], op=ALU.add)

    if stage <= 5:
        _bail(outacc.bitcast(F32)[:, 0:256])
        return

    # ---- out projection: per chunk transpose + matmul, then relu + store ----
    oT_ps = attT_oT[:, 512:1024]
    for c in range(CH):
        nc.tensor.transpose(
            oT_ps[:, 128 * c : 128 * (c + 1)],
            outacc[:, 128 * c : 128 * (c + 1)],
            identb[:, :],
        )
    oT = sbuf.tile([128, 512], BF16, tag="oTsb")
    nc.vector.tensor_copy(oT[:, :], oT_ps[:, :])
    y_ps = psS.tile([128, 256], F32, tag="ps")
    for c in range(CH):
        nc.tensor.matmul(
            y_ps[:, 64 * c : 64 * (c + 1)], oT[:, 128 * c : 128 * (c + 1)], wo4[:, :]
        )
    y_sb = sbuf.tile([128, 256], F32, tag="y_sb")
    nc.scalar.activation(y_sb[:, :], y_ps[:, :], AF.Relu)
    nc.sync.dma_start(
        y[b0 : b0 + 512, :].rearrange("(c p) f -> p c f", p=128), y_sb[:, :]
    )


def _build_v2(n_pad, stage=99):
    key = ("v2", n_pad, stage)
    if key in _BUILD_CACHE:
        return _BUILD_CACHE[key]
    assert n_pad % 512 == 0
    nc = bacc.Bacc()
    agent = nc.dram_tensor("agent", [n_pad, AGENT_SIZE], F32, kind="ExternalInput").ap()
    neigh = nc.dram_tensor(
        "neighbor", [n_pad, NK * NEIGH_SIZE], F32, kind="ExternalInput"
    ).ap()
    mask = nc.dram_tensor("mask", [n_pad, NK], I32, kind="ExternalInput").ap()
    wst_d = nc.dram_tensor("wst", [128, 384], BF16, kind="ExternalInput").ap()
    hsel_d = nc.dram_tensor("hsel", [128, 128], BF16, kind="ExternalInput").ap()
    wo4_d = nc.dram_tensor("wo4", [HM, OUT], BF16, kind="ExternalInput").ap()
    y = nc.dram_tensor("y", [n_pad, OUT], F32, kind="ExternalOutput").ap()

    with ExitStack() as ctx:
        tc = ctx.enter_context(tile.TileContext(nc))
        const = ctx.enter_context(tc.tile_pool(name="const", bufs=1))
        inp = ctx.enter_context(tc.tile_pool(name="inp", bufs=3))
        xtp = ctx.enter_context(tc.tile_pool(name="xtp", bufs=2))
        sbuf = ctx.enter_context(tc.tile_pool(name="sbuf", bufs=2))
        psA = ctx.enter_context(tc.tile_pool(name="psA", bufs=4, space="PSUM"))
        psT = ctx.enter_context(tc.tile_pool(name="psT", bufs=1, space="PSUM"))
        psS = ctx.enter_context(tc.tile_pool(name="psS", bufs=3, space="PSUM"))

        wst = const.tile([128, 384], BF16)
        nc.sync.dma_start(wst[:], wst_d[:, :])
        hsel4 = const.tile([128, 128], BF16)
        nc.sync.dma_start(hsel4[:], hsel_d[:, :])
        wo4 = const.tile([HM, OUT], BF16)
        nc.sync.dma_start(wo4[:], wo4_d[:, :])
        identb = const.tile([128, 128], BF16)
        masks.make_identity(nc, identb[:])

        pools = (inp, xtp, sbuf, psA, psT, psS)
        aps = (agent, neigh, mask, y, wst, hsel4, wo4, identb, None)
        for b in range(n_pad // 512):
            _emit_block_v2(tc, pools, aps, b * 512, False, stage=stage)

    nc.compile()
    _BUILD_CACHE[key] = nc
    return nc


def kernel(
    agent,
    neighbor,
    neighbor_mask,
    W_agent,
    b_agent,
    W_neigh,
    b_neigh,
    W_hid,
    b_hid,
    W_out,
    b_out,
    _trace=False,
):
    n = agent.shape[0]
    assert n % N_CORES == 0
    npc = n // N_CORES

    agent = np.ascontiguousarray(np.asarray(agent, dtype=np.float32))
    neighbor = np.ascontiguousarray(np.asarray(neighbor, dtype=np.float32)).reshape(n, NK * NEIGH_SIZE)
    neighbor_mask = np.ascontiguousarray(np.asarray(neighbor_mask, dtype=np.int32))

    biases = np.concatenate(
        [
            np.asarray(b_agent, np.float32).ravel(),
            np.asarray(b_neigh, np.float32).ravel(),
            np.asarray(b_hid, np.float32).ravel(),
            np.asarray(b_out, np.float32).ravel(),
        ]
    )[None, :]
    has_bias = bool(np.any(biases))
    use_v2 = (not has_bias) and os.environ.get("GAT_KERNEL_V1", "0") != "1"

    if use_v2:
        import ml_dtypes

        bf16 = ml_dtypes.bfloat16
        npad = ((npc + 511) // 512) * 512
        nc = _build_v2(npad)
        wa = np.asarray(W_agent, np.float32)
        wn = np.asarray(W_neigh, np.float32)
        wh = np.asarray(W_hid, np.float32)
        # stacked pair weights [128, 384] = [WaWa | WnWn | WhWh]
        wst = np.concatenate(
            [
                np.concatenate([wa, wa], axis=0),
                np.concatenate([wn, wn], axis=0),
                np.concatenate([wh, wh], axis=0),
            ],
            axis=1,
        ).astype(bf16)
        hsel = np.zeros((128, 128), np.float32)
        for j in range(4):
            for h in range(4):
                hsel[h * 32 : (h + 1) * 32, 32 * j + h] = 1.0
        wmaps = {
            "wst": wst,
            "hsel": hsel.astype(bf16),
            "wo4": (np.asarray(W_out, np.float32) / 4.0).astype(bf16),
        }
        pad = npad - npc
        in_maps = []
        for i in range(N_CORES):
            sl = slice(i * npc, (i + 1) * npc)
            m = {
                "agent": np.pad(agent[sl], ((0, pad), (0, 0))),
                "neighbor": np.pad(neighbor[sl], ((0, pad), (0, 0))),
                "mask": np.pad(neighbor_mask[sl], ((0, pad), (0, 0))),
                **wmaps,
            }
            in_maps.append(m)
        res = run_bass_kernel_spmd(nc, in_maps, list(range(N_CORES)), trace=_trace)
        out = np.concatenate(
            [res.results[i]["y"][:npc] for i in range(N_CORES)], axis=0
        )
        if _trace:
            kernel._last_results = res
        return out

    nc = _build(npc, has_bias)

    wmaps = {
        "wa": np.asarray(W_agent, np.float32),
        "wn": np.asarray(W_neigh, np.float32),
        "wh": np.asarray(W_hid, np.float32),
        "wo4": np.asarray(W_out, np.float32) / 4.0,
        "biases": biases.astype(np.float32),
    }
    in_maps = []
    for i in range(N_CORES):
        sl = slice(i * npc, (i + 1) * npc)
        in_maps.append(
            {
                "agent": agent[sl],
                "neighbor": neighbor[sl],
                "mask": neighbor_mask[sl],
                **wmaps,
            }
        )

    res = run_bass_kernel_spmd(nc, in_maps, list(range(N_CORES)), trace=_trace)
    out = np.concatenate([res.results[i]["y"] for i in range(N_CORES)], axis=0)
    if _trace:
        kernel._last_results = res
    return out



# revision 48
# speedup vs baseline: 1.6190x; 1.6190x over previous
"""GAT layer (nn_GATLayer_53841710022826) Bass/Tile kernel for Trainium2.

Data-parallel across 8 NeuronCores: each core processes a contiguous shard of
the agent dimension.  All weights are replicated.

Math (per agent n, heads h=4, mid m=32, neighbors k=4):
  a  = relu(agent @ W_agent + b_agent)            [n, 4, 32]
  nr = relu(neighbor @ W_neigh + b_neigh)         [n, 4, 4, 32]
  nh = relu(neighbor @ W_hid + b_hid)             [n, 4, 4, 32]
  att[n,h,k]   = sum_m a[n,h,m] * nr[n,k,h,m]
  score        = softmax_k(att - 1e8*mask)        (exactly 0 for masked k)
  out[n,h,m]   = sum_k score[n,h,k] * nh[n,k,h,m] / 4
  y  = relu(out @ W_out + b_out)                  [n, 64]

Kernel layout choice: agents on the partition dimension everywhere.  The
PE transposes each activation chunk ([P,64] -> [64,P]) so the transposed
activations act as the (self-loading f32) stationary matmul operand with the
small weight matrices as the moving operand; projections land in PSUM in
agent-major layout, where relu/attention/softmax are all free-dim vector ops.
"""

import os
import sys

if "/opt/trn_rl_repo" not in sys.path:
    sys.path.insert(0, "/opt/trn_rl_repo")

from contextlib import ExitStack

import numpy as np

import concourse.bass as bass
import concourse.tile as tile
from concourse import bacc, masks, mybir
from concourse.bass_utils import run_bass_kernel_spmd

F32 = mybir.dt.float32
I32 = mybir.dt.int32
AF = mybir.ActivationFunctionType
ALU = mybir.AluOpType
AX = mybir.AxisListType

N_CORES = 8
AGENT_SIZE = 64
NEIGH_SIZE = 64
NUM_HEAD = 4
MID_SIZE = 32
NK = 4
HM = NUM_HEAD * MID_SIZE  # 128
OUT = HM // 2  # 64

_BUILD_CACHE = {}


def _emit_chunk(ctx, tc, pools, aps, c0, P, has_bias):
    """Emit instructions for one chunk of P (<=128) agents starting at row c0."""
    nc = tc.nc
    (inp, xt_ps, xt_sb, proj_ps, work, out_ps, const) = pools
    (agent, neigh, mask, y, wa, wn, wh, wo4, bias_abc, bias_o, ident) = aps

    # ---- load inputs (f32) ----
    ag = inp.tile([128, AGENT_SIZE], F32, tag="ag")
    nc.sync.dma_start(ag[:P], agent[c0 : c0 + P, :])
    nb = inp.tile([128, NK * NEIGH_SIZE], F32, tag="nb")
    nc.sync.dma_start(nb[:P], neigh[c0 : c0 + P, :])
    mk = inp.tile([128, NK], I32, tag="mk")
    nc.sync.dma_start(mk[:P], mask[c0 : c0 + P, :])

    # ---- transpose activations: [P, 64] -> [64, P] (PE + PSUM->SBUF copy) ----
    # One PSUM bank tile holds agentT + neighT slots 0..2, a second holds slot 3.
    t1 = xt_ps.tile([64, 512], F32, tag="t1")
    t2 = xt_ps.tile([64, 128], F32, tag="t2")
    nc.tensor.transpose(t1[:, 0:P], ag[:P, :], ident[:P, :P])
    for k in range(3):
        nc.tensor.transpose(
            t1[:, 128 * (k + 1) : 128 * (k + 1) + P],
            nb[:P, 64 * k : 64 * (k + 1)],
            ident[:P, :P],
        )
    nc.tensor.transpose(t2[:, 0:P], nb[:P, 64 * 3 : 64 * 4], ident[:P, :P])

    xt = xt_sb.tile([64, 640], F32, tag="xt")  # [agT | nbT0..3]
    nc.scalar.activation(xt[:, 0:512], t1[:, :], AF.Copy)
    nc.scalar.activation(xt[:, 512:640], t2[:, :], AF.Copy)
    agT = lambda: xt[:, 0:P]
    nbT = lambda k: xt[:, 128 * (k + 1) : 128 * (k + 1) + P]

    # ---- projections: out[P, 128] = x @ W  (lhsT = xT, rhs = W) ----
    pa = proj_ps.tile([128, 512], F32, tag="pa")  # [a | nr0 | nr1 | nr2]
    pb = proj_ps.tile([128, 512], F32, tag="pb")  # [nr3 | nh0 | nh1 | nh2]
    pc = out_ps.tile([128, 320], F32, tag="pc")  # [nh3 | oT | y]
    nc.tensor.matmul(pa[:P, 0:128], agT(), wa[:, :])
    for k in range(4):
        dst = pa[:P, 128 * (k + 1) : 128 * (k + 2)] if k < 3 else pb[:P, 0:128]
        nc.tensor.matmul(dst, nbT(k), wn[:, :])
    for k in range(4):
        dst = pb[:P, 128 * (k + 1) : 128 * (k + 2)] if k < 3 else pc[:P, 0:128]
        nc.tensor.matmul(dst, nbT(k), wh[:, :])

    a_psv = pa[:P, 0:128]
    nr_psv = [
        pa[:P, 128:256],
        pa[:P, 256:384],
        pa[:P, 384:512],
        pb[:P, 0:128],
    ]
    nh_psv = [
        pb[:P, 128:256],
        pb[:P, 256:384],
        pb[:P, 384:512],
        pc[:P, 0:128],
    ]

    # ---- relu (PSUM -> SBUF); pa read only by DVE, pb/pc only by ACT ----
    # (single reader engine per PSUM tile keeps matmul sync-wait counts low)
    ba = bias_abc
    a_r = work.tile([128, HM], F32, tag="a_r")
    nr_r = work.tile([128, 4 * HM], F32, tag="nr_r")
    nh_r = work.tile([128, 4 * HM], F32, tag="nh_r")
    if has_bias:
        # out = relu(in * 1 + bias); ACT bias is per-partition, so biases are
        # handled with tensor_tensor add + max instead.
        nc.vector.tensor_tensor(a_psv, a_psv, ba[:P, 0:128], op=ALU.add)
        for k in range(4):
            nc.vector.tensor_tensor(nr_psv[k], nr_psv[k], ba[:P, 128:256], op=ALU.add)
            nc.vector.tensor_tensor(nh_psv[k], nh_psv[k], ba[:P, 256:384], op=ALU.add)
    nc.vector.tensor_scalar_max(a_r[:P], a_psv, 0.0)
    for k in range(4):
        dst_nr = nr_r[:P, 128 * k : 128 * (k + 1)]
        dst_nh = nh_r[:P, 128 * k : 128 * (k + 1)]
        if k < 3:
            nc.vector.tensor_scalar_max(dst_nr, nr_psv[k], 0.0)
            nc.scalar.activation(dst_nh, nh_psv[k], AF.Relu)
        else:
            nc.scalar.activation(dst_nr, nr_psv[k], AF.Relu)
            nc.scalar.activation(dst_nh, nh_psv[k], AF.Relu)

    # ---- attention logits: att[:, 4h+k] = sum_m a_r[:, hm] * nr_r_k[:, hm] ----
    att = work.tile([128, NUM_HEAD * NK], F32, tag="att")
    prod = work.tile([128, HM], F32, tag="prod")
    att_v = att[:P].rearrange("p (h k) -> p h k", k=NK)
    for k in range(4):
        nc.vector.tensor_tensor(
            prod[:P], a_r[:P], nr_r[:P, 128 * k : 128 * (k + 1)], op=ALU.mult
        )
        nc.vector.tensor_reduce(
            att_v[:, :, k],
            prod[:P].rearrange("p (h m) -> p h m", h=NUM_HEAD),
            axis=AX.X,
            op=ALU.add,
        )

    # ---- mask penalty + softmax over k (free-dim ops, agents on partitions) ----
    mkp = work.tile([128, NK], F32, tag="mkp")
    nc.vector.tensor_scalar_mul(mkp[:P], mk[:P], -1.0e8)
    attm = work.tile([128, NUM_HEAD * NK], F32, tag="attm")
    # broadcast mask penalty over heads: mkp [P, k] -> [P, h, k]
    mkp_b = mkp[:P].unsqueeze(1).broadcast_to([P, NUM_HEAD, NK])
    nc.vector.tensor_tensor(
        attm[:P].rearrange("p (h k) -> p h k", k=NK), att_v, mkp_b, op=ALU.add
    )
    es = work.tile([128, NUM_HEAD * NK], F32, tag="es")
    nc.scalar.activation(es[:P], attm[:P], AF.Exp)
    s4 = work.tile([128, NUM_HEAD], F32, tag="s4")
    nc.vector.tensor_reduce(
        s4[:P], es[:P].rearrange("p (h k) -> p h k", k=NK), axis=AX.X, op=ALU.add
    )
    s4m = work.tile([128, NUM_HEAD], F32, tag="s4m")
    nc.vector.tensor_scalar_max(s4m[:P], s4[:P], 1.0e-30)
    r4 = work.tile([128, NUM_HEAD], F32, tag="r4")
    nc.vector.reciprocal(r4[:P], s4m[:P])
    score = work.tile([128, NUM_HEAD * NK], F32, tag="score")
    r4_b = r4[:P].unsqueeze(2).broadcast_to([P, NUM_HEAD, NK])
    nc.vector.tensor_tensor(
        score[:P].rearrange("p (h k) -> p h k", k=NK),
        es[:P].rearrange("p (h k) -> p h k", k=NK),
        r4_b,
        op=ALU.mult,
    )

    # ---- weighted sum over neighbors: outacc[:, hm] = sum_k score_k * nh_r_k ----
    wk01 = work.tile([128, HM], F32, tag="wk01")
    wk23 = work.tile([128, HM], F32, tag="wk23")
    wkt = work.tile([128, HM], F32, tag="wkt")
    outacc = work.tile([128, HM], F32, tag="outacc")
    sc_v = score[:P].rearrange("p (h k) -> p h k", k=NK)

    def score_k(k):
        # [P, h] -> broadcast over m -> [P, h, m]
        return sc_v[:, :, k].unsqueeze(2).broadcast_to([P, NUM_HEAD, MID_SIZE])

    def nh_k(k):
        return nh_r[:P, 128 * k : 128 * (k + 1)].rearrange(
            "p (h m) -> p h m", h=NUM_HEAD
        )

    ge = nc.gpsimd
    ge.tensor_tensor(
        wk01[:P].rearrange("p (h m) -> p h m", h=NUM_HEAD), score_k(0), nh_k(0), op=ALU.mult
    )
    ge.tensor_tensor(
        wkt[:P].rearrange("p (h m) -> p h m", h=NUM_HEAD), score_k(1), nh_k(1), op=ALU.mult
    )
    ge.tensor_tensor(wk01[:P], wk01[:P], wkt[:P], op=ALU.add)
    ge.tensor_tensor(
        wk23[:P].rearrange("p (h m) -> p h m", h=NUM_HEAD), score_k(2), nh_k(2), op=ALU.mult
    )
    ge.tensor_tensor(
        wkt[:P].rearrange("p (h m) -> p h m", h=NUM_HEAD), score_k(3), nh_k(3), op=ALU.mult
    )
    ge.tensor_tensor(wk23[:P], wk23[:P], wkt[:P], op=ALU.add)
    ge.tensor_tensor(outacc[:P], wk01[:P], wk23[:P], op=ALU.add)

    # ---- output projection: y = relu(outacc @ (W_out/4) + b_out) ----
    oT_ps = pc[:, 128:256]
    nc.tensor.transpose(oT_ps[:, 0:P], outacc[:P, :], ident[:P, :P])
    oT = work.tile([128, 128], F32, tag="oTsb")
    nc.scalar.activation(oT[:, 0:P], oT_ps[:, 0:P], AF.Copy)
    y_ps = pc[:, 256:320]
    nc.tensor.matmul(y_ps[:P], oT[:, 0:P], wo4[:, :])
    if has_bias:
        nc.vector.tensor_tensor(y_ps[:P], y_ps[:P], bias_o[:P, :], op=ALU.add)
    y_r = work.tile([128, OUT], F32, tag="y_r")
    nc.scalar.activation(y_r[:P], y_ps[:P], AF.Relu)
    nc.sync.dma_start(y[c0 : c0 + P, :], y_r[:P])


def _build(n_per_core, has_bias):
    key = (n_per_core, has_bias)
    if key in _BUILD_CACHE:
        return _BUILD_CACHE[key]

    nc = bacc.Bacc()
    agent = nc.dram_tensor("agent", [n_per_core, AGENT_SIZE], F32, kind="ExternalInput").ap()
    neigh = nc.dram_tensor(
        "neighbor", [n_per_core, NK * NEIGH_SIZE], F32, kind="ExternalInput"
    ).ap()
    mask = nc.dram_tensor("mask", [n_per_core, NK], I32, kind="ExternalInput").ap()
    wa = nc.dram_tensor("wa", [AGENT_SIZE, HM], F32, kind="ExternalInput").ap()
    wn = nc.dram_tensor("wn", [NEIGH_SIZE, HM], F32, kind="ExternalInput").ap()
    wh = nc.dram_tensor("wh", [NEIGH_SIZE, HM], F32, kind="ExternalInput").ap()
    wo4 = nc.dram_tensor("wo4", [HM, OUT], F32, kind="ExternalInput").ap()
    biases = nc.dram_tensor("biases", [1, 3 * HM + OUT], F32, kind="ExternalInput").ap()
    y = nc.dram_tensor("y", [n_per_core, OUT], F32, kind="ExternalOutput").ap()

    with ExitStack() as ctx:
        tc = ctx.enter_context(tile.TileContext(nc))
        const = ctx.enter_context(tc.tile_pool(name="const", bufs=1))
        inp = ctx.enter_context(tc.tile_pool(name="inp", bufs=3))
        xt_ps = ctx.enter_context(tc.tile_pool(name="xt_ps", bufs=2, space="PSUM"))
        xt_sb = ctx.enter_context(tc.tile_pool(name="xt_sb", bufs=2))
        proj_ps = ctx.enter_context(tc.tile_pool(name="proj_ps", bufs=1, space="PSUM"))
        work = ctx.enter_context(tc.tile_pool(name="work", bufs=2))
        out_ps = ctx.enter_context(tc.tile_pool(name="out_ps", bufs=2, space="PSUM"))

        # constants in SBUF
        ident = const.tile([128, 128], F32)
        masks.make_identity(nc, ident[:])
        wa_sb = const.tile([AGENT_SIZE, HM], F32)
        nc.sync.dma_start(wa_sb[:], wa[:, :])
        wn_sb = const.tile([NEIGH_SIZE, HM], F32)
        nc.sync.dma_start(wn_sb[:], wn[:, :])
        wh_sb = const.tile([NEIGH_SIZE, HM], F32)
        nc.sync.dma_start(wh_sb[:], wh[:, :])
        wo4_sb = const.tile([HM, OUT], F32)
        nc.sync.dma_start(wo4_sb[:], wo4[:, :])
        bias_abc = None
        bias_o = None
        if has_bias:
            bias_abc = const.tile([128, 3 * HM], F32)
            nc.sync.dma_start(
                bias_abc[:], biases[0:1, 0 : 3 * HM].broadcast_to([128, 3 * HM])
            )
            bias_o = const.tile([128, OUT], F32)
            nc.sync.dma_start(
                bias_o[:], biases[0:1, 3 * HM :].broadcast_to([128, OUT])
            )

        pools = (inp, xt_ps, xt_sb, proj_ps, work, out_ps, const)
        aps = (agent, neigh, mask, y, wa_sb, wn_sb, wh_sb, wo4_sb, bias_abc, bias_o, ident)

        n_full, rem = divmod(n_per_core, 128)
        for c in range(n_full):
            _emit_chunk(None, tc, pools, aps, c * 128, 128, has_bias)
        if rem:
            _emit_chunk(None, tc, pools, aps, n_full * 128, rem, has_bias)

    nc.compile()
    _BUILD_CACHE[key] = nc
    return nc


BF16 = mybir.dt.bfloat16


def _emit_block_v2(tc, pools, aps, b0, has_bias, stage=99):
    """One block of 512 agents (4 chunks of 128), bf16 compute.

    Layouts:
      T-layout  [feature/hm on partitions, agents on free]  — projections, att
      A-layout  [agents on partitions, features on free]    — softmax, weighted sum
    """
    nc = tc.nc
    (inp, xtp, sbuf, psA, psT, psS) = pools
    (agent, neigh, mask, y, wst, hsel4, wo4, identb, maskc) = aps
    # wst: [128, 384] bf16 = [WaWa | WnWn | WhWh] stacked pairs;  wo4: [128, 64] bf16
    # hsel4: [128, 128] bf16 head-selector (col 32j+h = 1 for rows h*32..h*32+31 when j==k)
    CH = 4  # chunks per block

    # ---- load (cast f32 -> bf16 on SWDGE), block rows b0 .. b0+512 ----
    ag = inp.tile([128, CH * 64], BF16, tag="ag")
    nb = inp.tile([128, CH * 256], BF16, tag="nb")
    mk = inp.tile([128, CH * NK], I32, tag="mk")
    # DMA with 3D APs: partition = within-chunk row, free = (chunk, feat)
    nc.gpsimd.dma_start(
        ag[:, :], agent[b0 : b0 + 512, :].rearrange("(c p) f -> p c f", p=128)
    )
    nc.gpsimd.dma_start(
        nb[:, :], neigh[b0 : b0 + 512, :].rearrange("(c p) f -> p c f", p=128)
    )
    nc.sync.dma_start(
        mk[:, :], mask[b0 : b0 + 512, :].rearrange("(c p) k -> p c k", p=128)
    )

    # ---- input transposes on the PE (identity matmul) + PSUM->SBUF copies ----
    # (xbar dma_start_transpose costs ~1.2us/op on the Sync queue - it was the
    # kernel's critical path; PE transposes stream at ~107ns/tile instead)
    # xt_n01[:, 128c:...] rows 0-63 = k0 feats of chunk c, rows 64-127 = k1
    xt_n01 = xtp.tile([128, 512], BF16, tag="xt01")
    xt_n23 = xtp.tile([128, 512], BF16, tag="xt23")
    xt_a = xtp.tile([128, 256], BF16, tag="xta")
    t_ps1 = psT.tile([128, 1024], BF16, tag="pt", name="t_ps1")  # n01 | n23
    for c in range(CH):
        nc.tensor.transpose(
            t_ps1[:, 128 * c : 128 * (c + 1)], nb[:, 256 * c : 256 * c + 128], identb[:, :]
        )
        nc.tensor.transpose(
            t_ps1[:, 512 + 128 * c : 640 + 128 * c],
            nb[:, 256 * c + 128 : 256 * c + 256],
            identb[:, :],
        )
    nc.vector.tensor_copy(xt_n01[:, :], t_ps1[:, 0:512])
    nc.vector.tensor_copy(xt_n23[:, :], t_ps1[:, 512:1024])
    t_ps2 = psT.tile([128, 256], BF16, tag="pt", name="t_ps2")  # agent
    nc.tensor.transpose(t_ps2[:, 0:128], ag[:, 0:128], identb[:, :])
    nc.tensor.transpose(t_ps2[:, 128:256], ag[:, 128:256], identb[:, :])
    nc.scalar.activation(xt_a[:, :], t_ps2[:, :], AF.Copy)

    def _bail(t):
        w = t.shape[-1]
        y_sb = sbuf.tile([128, 256], F32, tag="y_sb")
        if w < 256:
            nc.gpsimd.memset(y_sb[:, :], 0.0)
        nc.vector.tensor_copy(y_sb[:, 0:w], t)
        nc.sync.dma_start(
            y[b0 : b0 + 512, :].rearrange("(c p) f -> p c f", p=128), y_sb[:, :]
        )

    if stage <= 1:
        _bail(xt_n01.bitcast(F32)[:, 0:256])
        return

    # ---- projections ----
    # T-layout: a [128hm, agents]; nr_k [128hm, 512a]
    # Concurrent row-tiled matmul pairs must write DIFFERENT psum banks
    # (same-bank column-disjoint concurrent PE writes conflict), so the agent
    # projection's top/bottom halves get separate tiles: a0 = (c0, c2),
    # a1 = (c1, c3).
    a_ps0 = psS.tile([128, 256], F32, tag="ps", name="a_ps0")
    a_ps1 = psS.tile([128, 256], F32, tag="ps", name="a_ps1")
    nr_ps = [psA.tile([128, 512], F32, tag="pp", name=f"nr_ps{k}") for k in range(NK)]
    for u in range(2):  # units (c0,c1), (c2,c3)
        nc.tensor.matmul(
            a_ps0[:, 128 * u : 128 * (u + 1)],
            wst[0:64, 0:128],
            xt_a[0:64, 128 * u : 128 * (u + 1)],
            tile_position=(0, 0),
        )
        nc.tensor.matmul(
            a_ps1[:, 128 * u : 128 * (u + 1)],
            wst[64:128, 0:128],
            xt_a[64:128, 128 * u : 128 * (u + 1)],
            tile_position=(64, 0),
        )

    # neighbors nr_k (T-layout), pairs (k0,k1) and (k2,k3) concurrently
    for kp in range(2):  # pair index
        xt = xt_n01 if kp == 0 else xt_n23
        nc.tensor.matmul(
            nr_ps[2 * kp][:, :], wst[0:64, 128:256], xt[0:64, :], tile_position=(0, 0)
        )
        nc.tensor.matmul(
            nr_ps[2 * kp + 1][:, :],
            wst[64:128, 128:256],
            xt[64:128, :],
            tile_position=(64, 0),
        )

    if stage <= 2:
        _bail(nr_ps[0][:, 0:256])
        return

    # ---- a_r = relu(a_ps) (ACT), prod_k = relu(nr_k) * a_r (DVE fused) ----
    a_r = sbuf.tile([128, 512], BF16, tag="a_r")
    a_r_v = a_r.rearrange("p (u c f) -> p u c f", u=2, c=2)  # (unit, chunk-in-unit, f)
    nc.scalar.activation(
        a_r_v.transpose([0, 2, 1, 3])[:, 0], a_ps0.rearrange("p (u f) -> p u f", u=2), AF.Relu
    )
    nc.scalar.activation(
        a_r_v.transpose([0, 2, 1, 3])[:, 1], a_ps1.rearrange("p (u f) -> p u f", u=2), AF.Relu
    )
    prods = []
    for k in range(NK):
        p_t = sbuf.tile([128, 512], BF16, tag=f"prod{k}")
        if k < 2:
            nc.vector.scalar_tensor_tensor(
                p_t[:, :], nr_ps[k][:, :], 0.0, a_r[:, :], op0=ALU.max, op1=ALU.mult
            )
        else:
            # split load: ACT relu + DVE mul
            nr_r = sbuf.tile([128, 512], BF16, tag=f"nr_r{k}")
            nc.scalar.activation(nr_r[:, :], nr_ps[k][:, :], AF.Relu)
            nc.vector.tensor_tensor(p_t[:, :], nr_r[:, :], a_r[:, :], op=ALU.mult)
        prods.append(p_t)

    # ---- attention logits via PE: att_ps rows 32j+h = att(k=j, h); cols agents ----
    att_ps = psS.tile([128, 512], F32, tag="ps")
    for k in range(NK):
        # M=32 (only cols 32k+h, h<4 are nonzero) so every PSUM row is written
        nc.tensor.matmul(
            att_ps[32 * k : 32 * k + 32, :],
            hsel4[:, 32 * k : 32 * k + 32],
            prods[k][:, :],
            tile_position=(0, 32 * k),
        )
    if stage <= 3:
        _bail(att_ps[:, 0:256])
        return

    att_sb = sbuf.tile([128, 512], BF16, tag="attsb")
    nc.scalar.activation(att_sb[:, :], att_ps[:, :], AF.Copy)

    # ---- transpose att to A-layout: attT[128a, (c, 32k+h)] ----
    attT_oT = psS.tile([128, 1024], BF16, tag="ps")
    attT = attT_oT[:, 0:512]
    for c in range(CH):
        nc.tensor.transpose(
            attT[:, 128 * c : 128 * (c + 1)],
            att_sb[:, 128 * c : 128 * (c + 1)],
            identb[:, :],
        )

    # ---- softmax over k (A-layout; all small free dims) ----
    mkp = sbuf.tile([128, CH * NK], F32, tag="mkp")
    nc.vector.tensor_scalar_mul(mkp[:, :], mk[:, :], -1.0e8)
    am = sbuf.tile([128, CH * 16], F32, tag="am")
    # iterate (c, k, h): in att at col [128c + 32k + h], out at (c, h, k)
    in_v = attT.rearrange("p (c r) -> p c r", c=CH)
    in_ckh = in_v.rearrange("p c (k r) -> p c k r", k=NK)[:, :, :, 0:4]
    mkp_ckh = mkp.rearrange("p (c k) -> p c k", c=CH).unsqueeze(3).broadcast_to(
        [128, CH, NK, NUM_HEAD]
    )
    am_ckh = am.rearrange("p (c h k) -> p c h k", c=CH, h=NUM_HEAD).transpose(
        [0, 1, 3, 2]
    )
    nc.vector.tensor_tensor(am_ckh, in_ckh, mkp_ckh, op=ALU.add)
    es = sbuf.tile([128, CH * 16], F32, tag="es")
    nc.scalar.activation(es[:, :], am[:, :], AF.Exp)
    ssum = sbuf.tile([128, CH * NUM_HEAD], F32, tag="ssum")
    nc.vector.tensor_reduce(
        ssum.rearrange("p (c h) -> p c h", c=CH),
        es.rearrange("p (c h k) -> p c h k", c=CH, h=NUM_HEAD),
        axis=AX.X,
        op=ALU.add,
    )
    rs = sbuf.tile([128, CH * NUM_HEAD], F32, tag="rs")
    nc.vector.tensor_scalar_max(ssum[:, :], ssum[:, :], 1.0e-30)
    nc.vector.reciprocal(rs[:, :], ssum[:, :])
    score = sbuf.tile([128, CH * 16], BF16, tag="score")
    nc.vector.tensor_tensor(
        score.rearrange("p (c h k) -> p c h k", c=CH, h=NUM_HEAD),
        es.rearrange("p (c h k) -> p c h k", c=CH, h=NUM_HEAD),
        rs.rearrange("p (c h) -> p c h", c=CH).unsqueeze(3).broadcast_to(
            [128, CH, NUM_HEAD, NK]
        ),
        op=ALU.mult,
    )

    if stage <= 4:
        _bail(es[:, :])
        return

    # ---- nh projections in A-layout: lhsT = xt unit (k-pair), rhs = Wh ----
    nh_ps = [psA.tile([128, 512], F32, tag="pp", name=f"nh_ps{k}") for k in range(NK)]
    for c in range(CH):
        for kp in range(2):
            xt = xt_n01 if kp == 0 else xt_n23
            nc.tensor.matmul(
                nh_ps[2 * kp][:, 128 * c : 128 * (c + 1)],
                xt[0:64, 128 * c : 128 * (c + 1)],
                wst[0:64, 256:384],
                tile_position=(0, 0),
            )
            nc.tensor.matmul(
                nh_ps[2 * kp + 1][:, 128 * c : 128 * (c + 1)],
                xt[64:128, 128 * c : 128 * (c + 1)],
                wst[64:128, 256:384],
                tile_position=(64, 0),
            )

    # ---- weighted sum: wk_k = relu(nh_k) * score_k (A-layout, fused) ----
    wks = []
    for k in range(NK):
        wk = sbuf.tile([128, 512], BF16, tag=f"wk{k}")
        sc_v = (
            score.rearrange("p (c h k) -> p c h k", c=CH, h=NUM_HEAD)[:, :, :, k]
            .unsqueeze(3)
            .broadcast_to([128, CH, NUM_HEAD, MID_SIZE])
        )
        nh_v = nh_ps[k].rearrange("p (c h m) -> p c h m", c=CH, h=NUM_HEAD)
        wk_v = wk.rearrange("p (c h m) -> p c h m", c=CH, h=NUM_HEAD)
        if k < 2:
            nc.vector.scalar_tensor_tensor(
                wk_v, nh_v, 0.0, sc_v, op0=ALU.max, op1=ALU.mult
            )
        else:
            nh_r = sbuf.tile([128, 512], BF16, tag=f"nh_r{k}")
            nc.scalar.activation(nh_r[:, :], nh_ps[k][:, :], AF.Relu)
            nc.gpsimd.tensor_tensor(
                wk_v, nh_r.rearrange("p (c h m) -> p c h m", c=CH, h=NUM_HEAD), sc_v,
                op=ALU.mult,
            )
        wks.append(wk)

    u01 = sbuf.tile([128, 512], BF16, tag="u01")
    u23 = sbuf.tile([128, 512], BF16, tag="u23")
    outacc = sbuf.tile([128, 512], BF16, tag="outacc")
    nc.gpsimd.tensor_tensor(u01[:, :], wks[0][:, :], wks[1][:, :], op=ALU.add)
    nc.gpsimd.tensor_tensor(u23[:, :], wks[2][:, :], wks[3][:, :], op=ALU.add)
    nc.gpsimd.tensor_tensor(outacc[:, :], u01[:, :], u23[:, :], op=ALU.add)

    if stage <= 5:
        _bail(outacc.bitcast(F32)[:, 0:256])
        return

    # ---- out projection: per chunk transpose + matmul, then relu + store ----
    oT_ps = attT_oT[:, 512:1024]
    for c in range(CH):
        nc.tensor.transpose(
            oT_ps[:, 128 * c : 128 * (c + 1)],
            outacc[:, 128 * c : 128 * (c + 1)],
            identb[:, :],
        )
    oT = sbuf.tile([128, 512], BF16, tag="oTsb")
    nc.vector.tensor_copy(oT[:, :], oT_ps[:, :])
    y_ps = psS.tile([128, 256], F32, tag="ps")
    for c in range(CH):
        nc.tensor.matmul(
            y_ps[:, 64 * c : 64 * (c + 1)], oT[:, 128 * c : 128 * (c + 1)], wo4[:, :]
        )
    y_sb = sbuf.tile([128, 256], F32, tag="y_sb")
    nc.scalar.activation(y_sb[:, :], y_ps[:, :], AF.Relu)
    nc.sync.dma_start(
        y[b0 : b0 + 512, :].rearrange("(c p) f -> p c f", p=128), y_sb[:, :]
    )


def _build_v2(n_pad, stage=99):
    key = ("v2", n_pad, stage)
    if key in _BUILD_CACHE:
        return _BUILD_CACHE[key]
    assert n_pad % 512 == 0
    nc = bacc.Bacc()
    agent = nc.dram_tensor("agent", [n_pad, AGENT_SIZE], F32, kind="ExternalInput").ap()
    neigh = nc.dram_tensor(
        "neighbor", [n_pad, NK * NEIGH_SIZE], F32, kind="ExternalInput"
    ).ap()
    mask = nc.dram_tensor("mask", [n_pad, NK], I32, kind="ExternalInput").ap()
    wst_d = nc.dram_tensor("wst", [128, 384], BF16, kind="ExternalInput").ap()
    hsel_d = nc.dram_tensor("hsel", [128, 128], BF16, kind="ExternalInput").ap()
    wo4_d = nc.dram_tensor("wo4", [HM, OUT], BF16, kind="ExternalInput").ap()
    y = nc.dram_tensor("y", [n_pad, OUT], F32, kind="ExternalOutput").ap()

    with ExitStack() as ctx:
        tc = ctx.enter_context(tile.TileContext(nc))
        const = ctx.enter_context(tc.tile_pool(name="const", bufs=1))
        inp = ctx.enter_context(tc.tile_pool(name="inp", bufs=3))
        xtp = ctx.enter_context(tc.tile_pool(name="xtp", bufs=2))
        sbuf = ctx.enter_context(tc.tile_pool(name="sbuf", bufs=2))
        psA = ctx.enter_context(tc.tile_pool(name="psA", bufs=4, space="PSUM"))
        psT = ctx.enter_context(tc.tile_pool(name="psT", bufs=1, space="PSUM"))
        psS = ctx.enter_context(tc.tile_pool(name="psS", bufs=3, space="PSUM"))

        wst = const.tile([128, 384], BF16)
        nc.sync.dma_start(wst[:], wst_d[:, :])
        hsel4 = const.tile([128, 128], BF16)
        nc.sync.dma_start(hsel4[:], hsel_d[:, :])
        wo4 = const.tile([HM, OUT], BF16)
        nc.sync.dma_start(wo4[:], wo4_d[:, :])
        identb = const.tile([128, 128], BF16)
        masks.make_identity(nc, identb[:])

        pools = (inp, xtp, sbuf, psA, psT, psS)
        aps = (agent, neigh, mask, y, wst, hsel4, wo4, identb, None)
        for b in range(n_pad // 512):
            _emit_block_v2(tc, pools, aps, b * 512, False, stage=stage)

    nc.compile()
    _BUILD_CACHE[key] = nc
    return nc


def _emit_block_v3(tc, pools, aps, b, att_ps, islot, nbl, stage=99):
    """Phase-1 compute for one 512-agent block: loads, projections, att logits.

    T-layout throughout: features on partitions, agents on the free dim.
    b = global block index; islot = index within the (up to 4-block) group.
    """
    nc = tc.nc
    (inp, nbp, prp, sb, psA, psNR, psNH, psSCB, psS2, psY) = pools
    (ag2_d, nb01_d, nb23_d, pen_d, y_d, waP, wnP, whP, hsel4, ksel, bsel, sstat, wo4) = aps

    agT = inp.tile([64, 512], BF16, tag="agT")
    nb01 = nbp.tile([128, 512], BF16, tag="nb01")
    nb23 = nbp.tile([128, 512], BF16, tag="nb23")
    nc.sync.dma_start(agT[:, :], ag2_d[:, b * 512 : (b + 1) * 512])
    nc.sync.dma_start(nb01[:, :], nb01_d[:, b * 512 : (b + 1) * 512])
    nc.sync.dma_start(nb23[:, :], nb23_d[:, b * 512 : (b + 1) * 512])

    if stage <= 0:
        y_sb = sb.tile([64, 512], F32, tag="y_bail")
        nc.vector.tensor_copy(y_sb[:, :], nb01[0:64, 0:512])
        nc.sync.dma_start(y_d[:, b * 512 : (b + 1) * 512], y_sb[:, :])
        return (nb01, nb23, [])

    # agent projection: a_ps[:, j] = Wa^T @ agent[512b+j]  (one matmul: a
    # quadrant pair into one PSUM bank crashes the exec unit - concurrent
    # same-bank writes)
    a_ps = psA.tile([128, 512], F32, tag="a_ps")
    nc.tensor.matmul(a_ps[:, :], waP[0:64, :], agT[:, :], tile_position=(0, 0))
    a_r = sb.tile([128, 512], BF16, tag="a_r")
    nc.scalar.activation(a_r[:, :], a_ps[:, :], AF.Relu)

    if stage <= 1:
        y_sb = sb.tile([64, 512], F32, tag="y_bail")
        nc.vector.tensor_copy(y_sb[:, :], a_r[0:64, 0:512])
        nc.sync.dma_start(y_d[:, b * 512 : (b + 1) * 512], y_sb[:, :])
        return (nb01, nb23, [])

    # neighbor projections nr_k + fused relu*a products, feeding att matmuls.
    # GPSIMD can't read PSUM: k0/k1 fused on DVE; k2/k3 drain on ACT then
    # multiply on Pool (k2) / DVE (k3).
    prods = []
    for k in range(NK):
        nb_t = nb01 if k < 2 else nb23
        rows = slice(0, 64) if (k % 2 == 0) else slice(64, 128)
        nr_ps = psNR.tile([128, 512], F32, tag="nr")
        nc.tensor.matmul(
            nr_ps[:, :], wnP[rows, :], nb_t[rows, :], tile_position=(rows.start, 0)
        )
        p_t = prp.tile([128, 512], BF16, tag=f"prod{k}")
        if k < 2:
            nc.vector.scalar_tensor_tensor(
                p_t[:, :], nr_ps[:, :], 0.0, a_r[:, :], op0=ALU.max, op1=ALU.mult
            )
        else:
            nr_r = sb.tile([128, 512], BF16, tag=f"nr_r{k}")
            nc.scalar.activation(nr_r[:, :], nr_ps[:, :], AF.Relu)
            eng = nc.gpsimd if k == 2 else nc.vector
            eng.tensor_tensor(p_t[:, :], nr_r[:, :], a_r[:, :], op=ALU.mult)
        prods.append(p_t)

    if stage <= 2:
        y_sb = sb.tile([64, 512], F32, tag="y_bail")
        nc.vector.tensor_copy(y_sb[:, :], prods[0][0:64, 0:512])
        nc.sync.dma_start(y_d[:, b * 512 : (b + 1) * 512], y_sb[:, :])
    return (nb01, nb23, prods)


def _emit_group_softmax_v3(tc, pools, aps, g, att_ps, psATT, nbl, stage=99):
    """Group softmax over k for 8 blocks at once.

    att_ps rows are (k, i, h) = 32k + 4i + h.  pen128 is host-pre-broadcast
    so the penalty lands in one DMA; nonexistent trailing blocks are
    host-masked so every op runs full-tile.  The k-sum and the reciprocal
    k-broadcast both run on the PE (selector matmuls) because engines cannot
    mix SBUF base partitions.
    """
    nc = tc.nc
    (inp, nbp, prp, sb, psA, psNR, psNH, psSCB, psS2, psY) = pools
    (ag2_d, nb01_d, nb23_d, pen_d, y_d, waP, wnP, whP, hsel4, ksel, bsel, sstat, wo4) = aps

    pen = inp.tile([128, 512], BF16, tag="pen")
    nc.sync.dma_start(pen[:, :], pen_d[:, g * 512 : (g + 1) * 512])
    nc.vector.tensor_tensor(att_ps[:, :], att_ps[:, :], pen[:, :], op=ALU.add)
    es = sb.tile([128, 512], BF16, tag="es")
    nc.scalar.activation(es[:, :], att_ps[:, :], AF.Exp)
    # k-sum on the PE: s2_ps[4i+h, :] = sum_k es[32k+4i+h, :]
    s2_ps = psS2.tile([32, 512], F32, tag="s2")
    nc.tensor.matmul(s2_ps[:, :], ksel[:, :], es[:, :], tile_position=(0, 0))
    nc.vector.tensor_scalar_max(s2_ps[:, :], s2_ps[:, :], 1.0e-30)
    rs = sb.tile([32, 512], BF16, tag="rs")
    with nc.allow_low_precision(reason="bf16 softmax reciprocal is plenty"):
        nc.vector.reciprocal(rs[:, :], s2_ps[:, :])
    # k-broadcast of the reciprocal on the PE, into the retired att_ps bank
    rsb_ps = psATT.tile([128, 512], F32, tag="att")
    nc.tensor.matmul(rsb_ps[:, :], bsel[:, :], rs[:, :], tile_position=(0, 0))
    score = sb.tile([128, 512], BF16, tag="score")
    nc.vector.tensor_tensor(score[:, :], es[:, :], rsb_ps[:, :], op=ALU.mult)
    if stage == 3:
        y_sb = sb.tile([64, 512], F32, tag="y_bail")
        nc.vector.tensor_copy(y_sb[:, :], es[0:64, :])
        nc.sync.dma_start(y_d[:, (8 * g) * 512 : (8 * g + 1) * 512], y_sb[:, :])
    return score


def _emit_block_phase2_v3(tc, pools, aps, b, islot, score, nbs, nbl, stage=99):
    """Phase-2 for one block: PE score broadcast, nh projections, weighted
    sum (folded into 4 accumulating output-projection matmuls), store."""
    nc = tc.nc
    (inp, nbp, prp, sb, psA, psNR, psNH, psSCB, psS2, psY) = pools
    (ag2_d, nb01_d, nb23_d, pen_d, y_d, waP, wnP, whP, hsel4, ksel, bsel, sstat, wo4) = aps
    (nb01, nb23, _prods) = nbs

    if stage <= 3:
        return
    zs = []
    for k in range(NK):
        # scoreB rows (h, m) <- score row (32k + 4*islot + h): selector matmul
        scB = psSCB.tile([128, 512], F32, tag="scB")
        nc.tensor.matmul(
            scB[:, :],
            sstat[:, 128 * (4 * islot + k) : 128 * (4 * islot + k + 1)],
            score[:, :],
            tile_position=(0, 0),
        )
        if stage <= 4:
            continue
        nb_t = nb01 if k < 2 else nb23
        rows = slice(0, 64) if (k % 2 == 0) else slice(64, 128)
        nh_ps = psNH.tile([128, 512], F32, tag="nh")
        nc.tensor.matmul(
            nh_ps[:, :], whP[rows, :], nb_t[rows, :], tile_position=(rows.start, 0)
        )
        # engines may read only ONE operand from PSUM: drain one side first
        z_t = sb.tile([128, 512], BF16, tag=f"z{k}")
        if k < 2:
            scB_sb = sb.tile([128, 512], BF16, tag=f"scBsb{k}")
            nc.scalar.activation(scB_sb[:, :], scB[:, :], AF.Copy)
            nc.vector.scalar_tensor_tensor(
                z_t[:, :], nh_ps[:, :], 0.0, scB_sb[:, :], op0=ALU.max, op1=ALU.mult
            )
        else:
            nh_r = sb.tile([128, 512], BF16, tag=f"nh_r{k}")
            nc.scalar.activation(nh_r[:, :], nh_ps[:, :], AF.Relu)
            nc.vector.tensor_tensor(z_t[:, :], nh_r[:, :], scB[:, :], op=ALU.mult)
        zs.append(z_t)

    if stage <= 4:
        y_sb = sb.tile([64, 512], F32, tag="y_bail")
        nc.vector.tensor_copy(y_sb[:, :], score[0:64, :])
        nc.sync.dma_start(y_d[:, b * 512 : (b + 1) * 512], y_sb[:, :])
        return

    # weighted sum folded into the output projection via PSUM accumulation
    y_ps = psY.tile([64, 512], F32, tag="y_ps")
    for k in range(NK):
        nc.tensor.matmul(
            y_ps[:, :], wo4[:, :], zs[k][:, :], start=(k == 0), stop=(k == NK - 1)
        )
    y_r = sb.tile([64, 512], F32, tag="y_r")
    nc.scalar.activation(y_r[:, :], y_ps[:, :], AF.Relu)
    nc.sync.dma_start(y_d[:, b * 512 : (b + 1) * 512], y_r[:, :])


def _build_v3(n_pad, stage=99):
    key = ("v3", n_pad, stage)
    if key in _BUILD_CACHE:
        return _BUILD_CACHE[key]
    assert n_pad % 512 == 0
    NB = n_pad // 512
    nc = bacc.Bacc()
    ag2_d = nc.dram_tensor("agT", [64, NB * 512], BF16, kind="ExternalInput").ap()
    nb01_d = nc.dram_tensor("nb01", [128, NB * 512], BF16, kind="ExternalInput").ap()
    nb23_d = nc.dram_tensor("nb23", [128, NB * 512], BF16, kind="ExternalInput").ap()
    NG = (NB + 7) // 8
    pen_d = nc.dram_tensor("pen128", [128, NG * 512], BF16, kind="ExternalInput").ap()
    wa_d = nc.dram_tensor("waP", [128, HM], BF16, kind="ExternalInput").ap()
    wn_d = nc.dram_tensor("wnP", [128, HM], BF16, kind="ExternalInput").ap()
    wh_d = nc.dram_tensor("whP", [128, HM], BF16, kind="ExternalInput").ap()
    hsel_d = nc.dram_tensor("hsel4", [128, 256], BF16, kind="ExternalInput").ap()
    ksel_d = nc.dram_tensor("ksel", [128, 32], BF16, kind="ExternalInput").ap()
    bsel_d = nc.dram_tensor("bsel", [32, 128], BF16, kind="ExternalInput").ap()
    sstat_d = nc.dram_tensor("sstat", [128, 4096], BF16, kind="ExternalInput").ap()
    wo4_d = nc.dram_tensor("wo4", [HM, OUT], BF16, kind="ExternalInput").ap()
    y_d = nc.dram_tensor("yT", [64, NB * 512], F32, kind="ExternalOutput").ap()

    with ExitStack() as ctx:
        tc = ctx.enter_context(tile.TileContext(nc))
        const = ctx.enter_context(tc.tile_pool(name="const", bufs=1))
        inp = ctx.enter_context(tc.tile_pool(name="inp", bufs=3))
        # nb tiles are read again in phase2, i.e. they stay live across
        # phase1 of the NEXT group (software pipelining) -> deep pool
        nbp = ctx.enter_context(tc.tile_pool(name="nbp", bufs=17))
        # prod tiles live until the group's att matmuls at phase1 end
        prp = ctx.enter_context(tc.tile_pool(name="prp", bufs=9))
        sb = ctx.enter_context(tc.tile_pool(name="sb", bufs=2))
        psA = ctx.enter_context(tc.tile_pool(name="psA", bufs=1, space="PSUM"))
        psNR = ctx.enter_context(tc.tile_pool(name="psNR", bufs=2, space="PSUM"))
        psNH = ctx.enter_context(tc.tile_pool(name="psNH", bufs=1, space="PSUM"))
        psSCB = ctx.enter_context(tc.tile_pool(name="psSCB", bufs=1, space="PSUM"))
        psATT = ctx.enter_context(tc.tile_pool(name="psATT", bufs=1, space="PSUM"))
        psS2 = ctx.enter_context(tc.tile_pool(name="psS2", bufs=1, space="PSUM"))
        psY = ctx.enter_context(tc.tile_pool(name="psY", bufs=1, space="PSUM"))

        waP = const.tile([128, HM], BF16)
        nc.sync.dma_start(waP[:], wa_d[:, :])
        wnP = const.tile([128, HM], BF16)
        nc.sync.dma_start(wnP[:], wn_d[:, :])
        whP = const.tile([128, HM], BF16)
        nc.sync.dma_start(whP[:], wh_d[:, :])
        hsel4 = const.tile([128, 256], BF16)
        nc.sync.dma_start(hsel4[:], hsel_d[:, :])
        ksel = const.tile([128, 32], BF16)
        nc.sync.dma_start(ksel[:], ksel_d[:, :])
        bsel = const.tile([32, 128], BF16)
        nc.sync.dma_start(bsel[:], bsel_d[:, :])
        sstat = const.tile([128, 4096], BF16)
        nc.sync.dma_start(sstat[:], sstat_d[:, :])
        wo4 = const.tile([HM, OUT], BF16)
        nc.sync.dma_start(wo4[:], wo4_d[:, :])

        pools = (inp, nbp, prp, sb, psA, psNR, psNH, psSCB, psS2, psY)
        aps = (
            ag2_d, nb01_d, nb23_d, pen_d, y_d,
            waP, wnP, whP, hsel4, ksel, bsel, sstat, wo4,
        )

        # software-pipelined group loop (8 blocks per group):
        # phase1(G) ... phase2(G-1)
        groups = []
        b0 = 0
        while b0 < NB:
            groups.append((b0, min(8, NB - b0)))
            b0 += 8

        pending = None  # (score, [per-block nbs], b0, nbl)
        for b0, nbl in groups:
            att_ps = psATT.tile([128, 512], F32, tag="att")
            nbs_list = []
            for i in range(max(nbl, pending[3] if pending else 0)):
                if i < nbl:
                    nbs_list.append(
                        _emit_block_v3(tc, pools, aps, b0 + i, att_ps, i, nbl, stage)
                    )
                if pending is not None and i < pending[3] and stage > 2:
                    _emit_block_phase2_v3(
                        tc, pools, aps, pending[2] + i, i, pending[0],
                        pending[1][i], pending[3], stage,
                    )
            if stage <= 2:
                pending = None
                continue
            # att logits, k-major so each PSUM accumulation group over the
            # 8 blocks closes before the next one opens (one open group per
            # bank): att_ps row (32k + 4i + h) = sum_m prods[i][k][32h+m, :]
            nc_ = tc.nc
            for k in range(NK):
                for i in range(nbl):
                    nc_.tensor.matmul(
                        att_ps[32 * k : 32 * k + 32, :],
                        hsel4[:, 32 * i : 32 * (i + 1)],
                        nbs_list[i][2][k][:, :],
                        tile_position=(0, 32 * k),
                        start=(i == 0),
                        stop=(i == nbl - 1),
                    )
            score = _emit_group_softmax_v3(
                tc, pools, aps, b0 // 8, att_ps, psATT, nbl, stage
            )
            pending = (score, nbs_list, b0, nbl)
        if pending is not None and stage > 2:
            (p_score, p_nbs, p_b0, p_nbl) = pending
            for i in range(p_nbl):
                _emit_block_phase2_v3(
                    tc, pools, aps, p_b0 + i, i, p_score, p_nbs[i], p_nbl, stage
                )

    nc.compile()
    _BUILD_CACHE[key] = nc
    return nc


def _prep_v3_inputs(agent, neighbor, neighbor_mask, npc, npad):
    """Host-side restaging: pad, transpose to feature-major, cast to bf16."""
    import ml_dtypes

    bf16 = ml_dtypes.bfloat16
    NB = npad // 512
    n = agent.shape[0]
    maps = []
    for i in range(N_CORES):
        sl = slice(i * npc, (i + 1) * npc)
        pad = npad - npc
        ag = np.pad(agent[sl], ((0, pad), (0, 0)))
        nb = np.pad(neighbor[sl].reshape(npc, NK, 64), ((0, pad), (0, 0), (0, 0)))
        mk = np.pad(neighbor_mask[sl], ((0, pad), (0, 0)))
        NG = (NB + 7) // 8
        mext = np.ones((NG * 8 * 512, NK), np.float32)
        mext[:npc] = mk[:npc]
        mext[npc:npad] = 0.0
        arr = (-30000.0 * mext).reshape(NG, 8, 512, NK)
        t = arr.transpose(3, 1, 0, 2)  # [k, i, g, j]
        pen128 = np.broadcast_to(
            t[:, :, None, :, :], (NK, 8, NUM_HEAD, NG, 512)
        ).reshape(128, NG * 512)
        # agT[f, b*512+j] = agent[b*512+j, f]
        agT = ag.T.copy()
        # nb01[kk*64+f, b*512+j] = neighbor[b*512+j, kk, f]
        nbr = nb.reshape(NB, 512, NK, 64)
        nb01 = nbr[:, :, 0:2, :].transpose(2, 3, 0, 1).reshape(128, NB * 512)
        nb23 = nbr[:, :, 2:4, :].transpose(2, 3, 0, 1).reshape(128, NB * 512)
        maps.append(
            {
                "agT": np.ascontiguousarray(agT).astype(bf16),
                "nb01": np.ascontiguousarray(nb01).astype(bf16),
                "nb23": np.ascontiguousarray(nb23).astype(bf16),
                "pen128": np.ascontiguousarray(pen128).astype(bf16),
            }
        )
    return maps


def kernel(
    agent,
    neighbor,
    neighbor_mask,
    W_agent,
    b_agent,
    W_neigh,
    b_neigh,
    W_hid,
    b_hid,
    W_out,
    b_out,
    _trace=False,
):
    n = agent.shape[0]
    assert n % N_CORES == 0
    npc = n // N_CORES

    agent = np.ascontiguousarray(np.asarray(agent, dtype=np.float32))
    neighbor = np.ascontiguousarray(np.asarray(neighbor, dtype=np.float32)).reshape(n, NK * NEIGH_SIZE)
    neighbor_mask = np.ascontiguousarray(np.asarray(neighbor_mask, dtype=np.int32))

    biases = np.concatenate(
        [
            np.asarray(b_agent, np.float32).ravel(),
            np.asarray(b_neigh, np.float32).ravel(),
            np.asarray(b_hid, np.float32).ravel(),
            np.asarray(b_out, np.float32).ravel(),
        ]
    )[None, :]
    has_bias = bool(np.any(biases))
    kver = os.environ.get("GAT_KERNEL", "v3")
    use_v3 = (not has_bias) and kver == "v3"
    use_v2 = (not has_bias) and not use_v3 and kver != "v1"

    if use_v3:
        import ml_dtypes

        bf16 = ml_dtypes.bfloat16
        npad = ((npc + 511) // 512) * 512
        stage = int(os.environ.get("GAT_V3_STAGE", "99"))
        nc = _build_v3(npad, stage)
        # block-i att selector: slice [:, 32i:32i+32], col 4i+h picks head h
        hsel = np.zeros((128, 256), np.float32)
        for i in range(8):
            for h in range(NUM_HEAD):
                hsel[h * MID_SIZE : (h + 1) * MID_SIZE, 32 * i + 4 * i + h] = 1.0
        # k-sum selector: col 4i+h sums es rows {32k + 4i + h : k}
        kselw = np.zeros((128, 32), np.float32)
        # reciprocal k-broadcast selector: col 32k+4i+h reads rs row 4i+h
        bselw = np.zeros((32, 128), np.float32)
        # scoreB selectors: slice (4i+k): col (32h+m) reads score row 32k+4i+h
        sstatw = np.zeros((128, 4096), np.float32)
        for i in range(8):
            for h in range(NUM_HEAD):
                for k in range(NK):
                    kselw[32 * k + 4 * i + h, 4 * i + h] = 1.0
                    bselw[4 * i + h, 32 * k + 4 * i + h] = 1.0
                    c0 = 128 * (4 * i + k)
                    sstatw[32 * k + 4 * i + h, c0 + 32 * h : c0 + 32 * h + 32] = 1.0
        wmaps = {
            "waP": np.concatenate([W_agent, W_agent], axis=0).astype(bf16),
            "wnP": np.concatenate([W_neigh, W_neigh], axis=0).astype(bf16),
            "whP": np.concatenate([W_hid, W_hid], axis=0).astype(bf16),
            "hsel4": hsel.astype(bf16),
            "ksel": kselw.astype(bf16),
            "bsel": bselw.astype(bf16),
            "sstat": sstatw.astype(bf16),
            "wo4": (np.asarray(W_out, np.float32) / 4.0).astype(bf16),
        }
        in_maps = _prep_v3_inputs(agent, neighbor, neighbor_mask, npc, npad)
        for m in in_maps:
            m.update(wmaps)
        res = run_bass_kernel_spmd(nc, in_maps, list(range(N_CORES)), trace=_trace)
        out = np.concatenate(
            [
                np.ascontiguousarray(res.results[i]["yT"].T)[:npc]
                for i in range(N_CORES)
            ],
            axis=0,
        )
        if _trace:
            kernel._last_results = res
        return out

    if use_v2:
        import ml_dtypes

        bf16 = ml_dtypes.bfloat16
        npad = ((npc + 511) // 512) * 512
        nc = _build_v2(npad)
        wa = np.asarray(W_agent, np.float32)
        wn = np.asarray(W_neigh, np.float32)
        wh = np.asarray(W_hid, np.float32)
        # stacked pair weights [128, 384] = [WaWa | WnWn | WhWh]
        wst = np.concatenate(
            [
                np.concatenate([wa, wa], axis=0),
                np.concatenate([wn, wn], axis=0),
                np.concatenate([wh, wh], axis=0),
            ],
            axis=1,
        ).astype(bf16)
        hsel = np.zeros((128, 128), np.float32)
        for j in range(4):
            for h in range(4):
                hsel[h * 32 : (h + 1) * 32, 32 * j + h] = 1.0
        wmaps = {
            "wst": wst,
            "hsel": hsel.astype(bf16),
            "wo4": (np.asarray(W_out, np.float32) / 4.0).astype(bf16),
        }
        pad = npad - npc
        in_maps = []
        for i in range(N_CORES):
            sl = slice(i * npc, (i + 1) * npc)
            m = {
                "agent": np.pad(agent[sl], ((0, pad), (0, 0))),
                "neighbor": np.pad(neighbor[sl], ((0, pad), (0, 0))),
                "mask": np.pad(neighbor_mask[sl], ((0, pad), (0, 0))),
                **wmaps,
            }
            in_maps.append(m)
        res = run_bass_kernel_spmd(nc, in_maps, list(range(N_CORES)), trace=_trace)
        out = np.concatenate(
            [res.results[i]["y"][:npc] for i in range(N_CORES)], axis=0
        )
        if _trace:
            kernel._last_results = res
        return out

    nc = _build(npc, has_bias)

    wmaps = {
        "wa": np.asarray(W_agent, np.float32),
        "wn": np.asarray(W_neigh, np.float32),
        "wh": np.asarray(W_hid, np.float32),
        "wo4": np.asarray(W_out, np.float32) / 4.0,
        "biases": biases.astype(np.float32),
    }
    in_maps = []
    for i in range(N_CORES):
        sl = slice(i * npc, (i + 1) * npc)
        in_maps.append(
            {
                "agent": agent[sl],
                "neighbor": neighbor[sl],
                "mask": neighbor_mask[sl],
                **wmaps,
            }
        )

    res = run_bass_kernel_spmd(nc, in_maps, list(range(N_CORES)), trace=_trace)
    out = np.concatenate([res.results[i]["y"] for i in range(N_CORES)], axis=0)
    if _trace:
        kernel._last_results = res
    return out

